# revision 2
# baseline (speedup 1.0000x reference)
"""2-layer GCN (GCNConv x2 + ReLU) on 8 Trainium2 NeuronCores — v3.

Key measured constraints this design is built around:
  - SWDGE desc-gen on the Q7 is ~9ns/descriptor serial, ~2.6ns effective when
    calls are pipelined round-robin over 4 queues -> issue chunk k's gather
    pieces on queue k, all four chunks concurrently.
  - DVE ops stall ~10-30x while desc-gen runs -> keep DVE out of the gather
    phase entirely: S one-hot tiles are HOST-precomputed and streamed from
    DRAM via affine HWDGE DMA; scale/bias/relu run on the ACT engine; the
    bias and self-loop are folded into the PE accumulation chain.
  - fp16 matmuls run at fp32 rate -> everything bf16.
  - PSUM tiles are bank-granular (8 max) -> block-major chains: each block's
    4 chunk-segments are processed consecutively so only ~4 agg tiles live.

Layout: slots ordered (chunk, block), per-(block,chunk) groups padded to a
multiple of 128 so every 128-slot tile belongs to one block. agg_b(PSUM) =
sum_k sum_tiles S_t^T @ M_t + rank1(rdis_b x bias) + I @ hs_b, then
out = Relu(agg * dis_b) on ACT.
"""
import sys, os, time, types

sys.path.insert(0, '/opt/trn_rl_repo')
if 'antenv.axon_hooks' not in sys.modules:
    _m = types.ModuleType('antenv.axon_hooks')
    _m.get_axon_ntff_profile_hook = lambda: None
    sys.modules['antenv.axon_hooks'] = _m

import numpy as np
import ml_dtypes
import concourse.bass as bass
import concourse.bacc as bacc
import concourse.mybir as mybir
import concourse.tile as tile
from concourse import library_config
from concourse.masks import make_identity
from concourse.bass_utils import run_bass_kernel_spmd

P = 128
F32, BF16, I16 = mybir.dt.float32, mybir.dt.bfloat16, mybir.dt.int16
BF = ml_dtypes.bfloat16


class Cfg:
    def __init__(self, n_nodes, n_cores, n_chunks=4, cap=4096):
        self.N = n_nodes
        self.NC = n_cores
        self.SH = n_nodes // n_cores           # nodes per shard
        assert self.SH * n_cores == n_nodes
        self.NB = (self.SH + P - 1) // P       # dst blocks per shard
        self.PSH = self.NB * P                 # padded shard rows
        self.TBL = self.PSH * n_cores          # padded table rows
        assert self.TBL % n_chunks == 0
        self.CH = self.TBL // n_chunks         # chunk rows (int16-indexable)
        assert self.CH <= 32768
        self.NK = n_chunks
        self.CAP = cap                         # max idxs per dma_gather call
        assert cap % P == 0


def _route(cfg, edge_index):
    """Host-side routing: idx streams, S one-hot tiles, call/group schedule."""
    N, NC, SH, NB, PSH, CH, NK, CAP = (cfg.N, cfg.NC, cfg.SH, cfg.NB, cfg.PSH,
                                       cfg.CH, cfg.NK, cfg.CAP)
    src = np.asarray(edge_index[0], dtype=np.int64)
    dst = np.asarray(edge_index[1], dtype=np.int64)
    deg = (np.bincount(dst, minlength=N) + 1).astype(np.float32)

    core = dst // SH
    dl = dst - core * SH
    b = dl >> 7
    dloc = (dl & 127).astype(np.int16)
    r = (src // SH) * PSH + (src % SH)
    k = r // CH
    ri = (r % CH).astype(np.int16)

    gid = (k * NB + b)
    order = np.argsort((core * np.int64(NK * NB) + gid) * np.int64(CH) + ri,
                       kind='stable')
    gid_s, ri_s, dloc_s = gid[order], ri[order], dloc[order]
    core_s = core[order]
    sizes = np.zeros((NC, NK * NB), np.int64)
    for c in range(NC):
        m = core_s == c
        sizes[c] = np.bincount(gid_s[m], minlength=NK * NB)
    starts = np.zeros((NC, NK * NB + 1), np.int64)
    np.cumsum(sizes, axis=1, out=starts[:, 1:])
    base = np.concatenate([[0], np.cumsum(sizes.sum(axis=1))])[:-1]

    # per-(k,b) capacity: max over cores, round up to 128
    C = np.maximum(((sizes.max(axis=0) + 127) // 128) * 128, 128)  # [NK*NB]
    goffs = np.zeros(NK * NB + 1, np.int64)
    np.cumsum(C, out=goffs[1:])
    TOT = int(goffs[-1])
    TILES = TOT // P

    # call schedule: per chunk, pieces of <=CAP slots; queue = chunk
    calls = []        # (k, slot_off, n, tile_off, T, queue)
    for kk in range(NK):
        k0, k1 = int(goffs[kk * NB]), int(goffs[(kk + 1) * NB])
        off = k0
        while off < k1:
            n = min(CAP, k1 - off)
            calls.append((kk, off, n, off // P, n // P, kk % 4))
            off += n
    TMAX = max(c[4] for c in calls)

    # per-(k,b) tile ranges
    groups = []       # [(k, b, gt0, gt1)]
    for kk in range(NK):
        for bb in range(NB):
            g = kk * NB + bb
            groups.append((kk, bb, int(goffs[g]) // P, int(goffs[g + 1]) // P))

    # per-core idx and host-built S tiles
    idx_all = np.zeros((NC, TOT), np.int16)
    dloc_all = np.full((NC, TOT), -1, np.int16)
    for c in range(NC):
        for g in range(NK * NB):
            s0, s1 = base[c] + starts[c, g], base[c] + starts[c, g + 1]
            n = s1 - s0
            go = int(goffs[g])
            cap = int(C[g])
            if n > 0:
                idx_all[c, go:go + n] = ri_s[s0:s1]
                dloc_all[c, go:go + n] = dloc_s[s0:s1]
                if n < cap:
                    idx_all[c, go + n:go + cap] = ri_s[s1 - 1]

    idx16 = np.zeros((NC, 128, TOT // 16), np.int16)
    for c in range(NC):
        a = idx_all[c].reshape(TOT // 16, 16).T
        idx16[c] = np.tile(a, (8, 1))

    # S DRAM layout: [128 (slot-in-tile), TILES*128]; col block t holds S_t
    # S_t[p, d] = 1 if dloc(slot p of tile t) == d
    S_host = np.zeros((NC, 128, TILES * 128), BF)
    eye = np.eye(128, dtype=BF)
    for c in range(NC):
        dl2 = dloc_all[c].reshape(TILES, 128)
        St = np.zeros((TILES, 128, 128), BF)
        valid = dl2 >= 0
        ti, pi = np.nonzero(valid)
        St[ti, pi, dl2[ti, pi]] = 1.0
        S_host[c] = St.transpose(1, 0, 2).reshape(128, TILES * 128)

    degT = np.ones((NC, P, NB), np.float32)
    for c in range(NC):
        dd = np.ones(PSH, np.float32)
        dd[:SH] = deg[c * SH:(c + 1) * SH]
        degT[c] = dd.reshape(NB, P).T
    rdisT = np.sqrt(degT.transpose(0, 2, 1).reshape(NC, 1, PSH))  # [NC,1,PSH]
    return calls, groups, TOT, TILES, TMAX, idx16, S_host, degT, rdisT


def _build(cfg, calls, groups, TOT, TILES, TMAX, repeat=1):
    NB, PSH, CH, SH, NK = cfg.NB, cfg.PSH, cfg.CH, cfg.SH, cfg.NK
    nc = bacc.Bacc("TRN2", target_bir_lowering=False, debug=False,
                   num_devices=cfg.NC, num_swdge_queues=4)
    xT_d = nc.dram_tensor("xT", [P, PSH], BF16, kind="ExternalInput")
    degT_d = nc.dram_tensor("degT", [P, NB], F32, kind="ExternalInput")
    rdisT_d = nc.dram_tensor("rdisT", [1, PSH], BF16, kind="ExternalInput")
    idx_d = nc.dram_tensor("idx16", [P, TOT // 16], I16, kind="ExternalInput")
    S_d = nc.dram_tensor("Shost", [P, TILES * P], BF16, kind="ExternalInput")
    W1_d = nc.dram_tensor("W1", [P, P], BF16, kind="ExternalInput")
    W2_d = nc.dram_tensor("W2", [P, P], BF16, kind="ExternalInput")
    b1_d = nc.dram_tensor("b1", [1, P], BF16, kind="ExternalInput")
    b2_d = nc.dram_tensor("b2", [1, P], BF16, kind="ExternalInput")
    out_d = nc.dram_tensor("out", [SH, P], F32, kind="ExternalOutput")

    gt2call = {}
    for ci, (kk, soff, n, toff, T, q) in enumerate(calls):
        for lt in range(T):
            gt2call[toff + lt] = (ci, lt)
    # group list indexed by (k, b)
    gmap = {(kk, bb): (gt0, gt1) for kk, bb, gt0, gt1 in groups}

    ts = bass.ts
    with tile.TileContext(nc) as tc:
        with tc.tile_pool(name="const", bufs=1) as cpool, \
             tc.tile_pool(name="dram", bufs=1, space="DRAM") as dpool, \
             tc.tile_pool(name="msg", bufs=6) as mpool, \
             tc.tile_pool(name="sel", bufs=6) as spool, \
             tc.tile_pool(name="stream", bufs=8) as stpool, \
             tc.tile_pool(name="work", bufs=4) as wpool, \
             tc.tile_pool(name="mmp", bufs=2, space="PSUM") as mmpool, \
             tc.tile_pool(name="aggp", bufs=4, space="PSUM") as aggpool, \
             tc.tile_pool(name="trp", bufs=2, space="PSUM") as trpool:
            nc.gpsimd.load_library(library_config.mlp)
            degT = cpool.tile([P, NB], F32)
            rdisT = cpool.tile([1, PSH], BF16)
            W1s = cpool.tile([P, P], BF16)
            W2s = cpool.tile([P, P], BF16)
            b1s = cpool.tile([1, P], BF16)
            b2s = cpool.tile([1, P], BF16)
            for sb, dr in ((degT, degT_d), (rdisT, rdisT_d), (W1s, W1_d),
                           (W2s, W2_d), (b1s, b1_d), (b2s, b2_d)):
                nc.sync.dma_start(sb[:], dr[:])

            ident = cpool.tile([P, P], BF16)
            make_identity(nc, ident[:])

            dis = cpool.tile([P, NB], F32)
            nc.vector.reciprocal(dis[:], degT[:])
            nc.scalar.sqrt(dis[:], dis[:])

            hs_all = cpool.tile([P, NB * P], BF16)
            h1T = cpool.tile([P, PSH], BF16)

            rg = [list(range(cfg.NC))]

            def layer(Wsb, bsrc, src_in, src_full, is_last):
                # hs = (h @ W) * dis -> bf16 table shard (ACT does the scale)
                for t in range(NB):
                    ps = mmpool.tile([P, P], F32, tag="mm")
                    if not is_last:
                        xt = wpool.tile([P, P], BF16, tag="xt")
                        nc.sync.dma_start(xt[:], xT_d[:, ts(t, P)])
                        lhsT = xt[:]
                    else:
                        lhsT = h1T[:, ts(t, P)]
                    nc.tensor.matmul(ps[:], lhsT=lhsT, rhs=Wsb[:],
                                     start=True, stop=True)
                    nc.scalar.mul(hs_all[:, ts(t, P)], ps[:],
                                  dis[:, t:t + 1])
                nc.sync.dma_start(
                    src_in[:].rearrange("(t p) d -> p t d", p=P),
                    hs_all[:].rearrange("p (t d) -> p t d", d=P))
                nc.gpsimd.collective_compute(
                    "AllGather", mybir.AluOpType.bypass, replica_groups=rg,
                    ins=[src_in.opt()], outs=[src_full.opt()])

                mS = {}

                def ensure_call(ci):
                    if ci in mS or ci >= len(calls):
                        return
                    kk, soff, n, toff, T, q = calls[ci]
                    ix = stpool.tile([P, cfg.CAP // 16], I16, tag="ix")
                    nc.sync.dma_start(ix[:, :n // 16],
                                      idx_d[:, soff // 16:(soff + n) // 16])
                    S = spool.tile([P, TMAX * P], BF16, tag="sel")
                    nc.sync.dma_start(S[:, :T * P],
                                      S_d[:, toff * P:(toff + T) * P])
                    m = mpool.tile([P, TMAX, P], BF16, tag="msg")
                    nc.gpsimd.dma_gather(
                        m[:, :T, :], src_full[kk * CH:(kk + 1) * CH, :],
                        ix[:, :n // 16], n, n, P,
                        queue_num=q, single_packet=False)
                    mS[ci] = (m, S)

                # block-major chains; calls ensured lazily w/ lookahead
                for bb in range(NB):
                    # prefetch: ensure calls covering this and next block
                    for lb in (bb, min(bb + 1, NB - 1)):
                        for kk in range(NK):
                            gt0, gt1 = gmap[(kk, lb)]
                            for gt in (gt0, gt1 - 1):
                                ensure_call(gt2call[gt][0])
                    agg = aggpool.tile([P, P], F32, tag="agg")
                    nmm = sum(gmap[(kk, bb)][1] - gmap[(kk, bb)][0]
                              for kk in range(NK)) + 2
                    st = 0
                    for kk in range(NK):
                        gt0, gt1 = gmap[(kk, bb)]
                        for gt in range(gt0, gt1):
                            ci, lt = gt2call[gt]
                            ensure_call(ci)
                            m, S = mS[ci]
                            nc.tensor.matmul(agg[:],
                                             lhsT=S[:, lt * P:(lt + 1) * P],
                                             rhs=m[:, lt, :],
                                             start=(st == 0),
                                             stop=False)
                            st += 1
                    # bias: rank-1 (rdis_b ⊗ bias) so ACT can scale by dis
                    nc.tensor.matmul(agg[:], lhsT=rdisT[:, ts(bb, P)],
                                     rhs=bsrc[:], start=False, stop=False)
                    # self-loop message
                    nc.tensor.matmul(agg[:], lhsT=ident[:],
                                     rhs=hs_all[:, ts(bb, P)],
                                     start=False, stop=True)
                    if not is_last:
                        t2 = wpool.tile([P, P], BF16, tag="t2")
                        nc.scalar.activation(t2[:], agg[:],
                                             mybir.ActivationFunctionType.Relu,
                                             scale=dis[:, bb:bb + 1])
                        pT = trpool.tile([P, P], BF16, tag="pT")
                        nc.tensor.transpose(pT[:], t2[:], ident[:])
                        nc.scalar.copy(h1T[:, ts(bb, P)], pT[:])
                    else:
                        o = wpool.tile([P, P], F32, tag="o")
                        nc.scalar.activation(o[:], agg[:],
                                             mybir.ActivationFunctionType.Relu,
                                             scale=dis[:, bb:bb + 1])
                        rows = min(P, SH - bb * P)
                        nc.sync.dma_start(out_d[bb * P:bb * P + rows, :],
                                          o[:rows, :])

            for _rep in range(repeat):
                hs1_in = dpool.tile([PSH, P], BF16, name=f"hs1i{_rep}")
                hs1_full = dpool.tile([cfg.TBL, P], BF16, addr_space="Shared",
                                      name=f"hs1f{_rep}")
                hs2_in = dpool.tile([PSH, P], BF16, name=f"hs2i{_rep}")
                hs2_full = dpool.tile([cfg.TBL, P], BF16, addr_space="Shared",
                                      name=f"hs2f{_rep}")
                layer(W1s, b1s, hs1_in, hs1_full, False)
                layer(W2s, b2s, hs2_in, hs2_full, True)
    nc.compile()
    return nc


_CACHE = {}


def _prepare(cfg, x, edge_index, W1, b1, W2, b2):
    key = (int(os.environ.get("KREPEAT", "1")), cfg.N, cfg.NC, cfg.CAP,
           int(np.asarray(edge_index[0, :64]).sum()),
           int(np.asarray(edge_index).sum() % (1 << 62)))
    if key not in _CACHE:
        calls, groups, TOT, TILES, TMAX, idx16, S_host, degT, rdisT = \
            _route(cfg, edge_index)
        nc = _build(cfg, calls, groups, TOT, TILES, TMAX,
                    repeat=int(os.environ.get("KREPEAT", "1")))
        _CACHE[key] = (nc, idx16, S_host, degT, rdisT)
    nc, idx16, S_host, degT, rdisT = _CACHE[key]

    x = np.asarray(x, np.float32)
    in_maps = []
    for c in range(cfg.NC):
        xs = np.zeros((cfg.PSH, P), np.float32)
        xs[:cfg.SH] = x[c * cfg.SH:(c + 1) * cfg.SH]
        in_maps.append({
            "xT": np.ascontiguousarray(xs.T).astype(BF),
            "degT": degT[c],
            "rdisT": rdisT[c].astype(BF),
            "idx16": idx16[c],
            "Shost": S_host[c],
            "W1": np.asarray(W1, np.float32).astype(BF),
            "W2": np.asarray(W2, np.float32).astype(BF),
            "b1": np.asarray(b1, np.float32).reshape(1, P).astype(BF),
            "b2": np.asarray(b2, np.float32).reshape(1, P).astype(BF),
        })
    return nc, in_maps


_FAST = {}


def run_fast(cfg, x, edge_index, W1, b1, W2, b2):
    """Caches the jitted executable + device-resident inputs."""
    import jax
    from jax.sharding import Mesh, PartitionSpec
    from jax.experimental.shard_map import shard_map
    from concourse import bass2jax
    import concourse.mybir as mb

    nc, in_maps = _prepare(cfg, x, edge_index, W1, b1, W2, b2)
    key = id(nc)
    if key not in _FAST:
        bass2jax.install_neuronx_cc_hook()
        partition_name = (nc.partition_id_tensor.name
                          if nc.partition_id_tensor else None)
        in_names, out_names, out_avals = [], [], []
        for alloc in nc.m.functions[0].allocations:
            if not isinstance(alloc, mb.MemoryLocationSet):
                continue
            name = alloc.memorylocations[0].name
            if alloc.kind == "ExternalInput":
                if name != partition_name:
                    in_names.append(name)
            elif alloc.kind == "ExternalOutput":
                out_names.append(name)
                out_avals.append(jax.core.ShapedArray(
                    tuple(alloc.tensor_shape), mb.dt.np(alloc.dtype)))
        n_params = len(in_names)
        zero_outs = [np.zeros(a.shape, a.dtype) for a in out_avals]
        all_names = in_names + out_names + (
            [partition_name] if partition_name else [])

        def _body(*args):
            operands = list(args)
            if partition_name is not None:
                operands.append(bass2jax.partition_id_tensor())
            return tuple(bass2jax._bass_exec_p.bind(
                *operands, out_avals=tuple(out_avals),
                in_names=tuple(all_names), out_names=tuple(out_names),
                lowering_input_output_aliases=(),
                sim_require_finite=True, sim_require_nnan=True, nc=nc))

        devices = jax.devices()[:cfg.NC]
        mesh = Mesh(np.asarray(devices), ("core",))
        n_outs = len(out_names)
        fn = jax.jit(shard_map(
            _body, mesh=mesh,
            in_specs=(PartitionSpec("core"),) * (n_params + n_outs),
            out_specs=(PartitionSpec("core"),) * n_outs, check_rep=False),
            keep_unused=True)
        sharding = jax.sharding.NamedSharding(mesh, PartitionSpec("core"))
        dev_in = [jax.device_put(
            np.concatenate([in_maps[c][nm] for c in range(cfg.NC)], axis=0),
            sharding) for nm in in_names]
        dev_zero = [jax.device_put(
            np.zeros((cfg.NC * z.shape[0],) + z.shape[1:], z.dtype), sharding)
            for z in zero_outs]
        _FAST[key] = (fn, dev_in, dev_zero, out_names, out_avals)
    fn, dev_in, dev_zero, out_names, out_avals = _FAST[key]
    outs = fn(*dev_in, *dev_zero)
    jax.block_until_ready(outs)
    oi = out_names.index("out")
    o = np.asarray(outs[oi]).reshape(cfg.NC, *out_avals[oi].shape)
    return o.reshape(cfg.NC * out_avals[oi].shape[0], out_avals[oi].shape[1])


def run(cfg, x, edge_index, W1, b1, W2, b2, trace=False, tmpdir=None):
    nc, in_maps = _prepare(cfg, x, edge_index, W1, b1, W2, b2)
    res = run_bass_kernel_spmd(nc, in_maps, core_ids=list(range(cfg.NC)),
                               trace=trace, tmpdir=tmpdir)
    if trace:
        print("exec_time_ns:", res.exec_time_ns)
    return np.concatenate([r["out"] for r in res.results], axis=0)


def kernel(x, edge_index, W1, b1, W2, b2):
    cfg = Cfg(100000, 8)
    return run(cfg, x, edge_index, W1, b1, W2, b2)
